# revision 8
# baseline (speedup 1.0000x reference)
"""Trainium2 Bass kernel v4 for nn_MultiHeadAttention (B=2, T=2048, C=1024, H=16).

Sharding (8 cores): data-parallel over batch (2) x tensor-parallel over head
groups (4 groups of 4 heads), Megatron-style row-parallel c_proj; the 4
partials per batch are summed on the host. No device collectives.

Structure (per core):
  - All matmul operands bf16 (PSUM accumulation fp32).
  - Natural-layout AV (et stationary, v+ones moving, 65-row passes);
    denominator lands in free column 64; normalization = per-partition
    tensor_scalar multiply by reciprocal_approx_fast.
  - y transposed back to yT via DMA XBAR transposes (bf16), zero PE cost.
  - Manual PSUM layout (zero regions are 2KB banks):
      banks 0-3: two sc slots [128,1024] (QK out / exp in), alternating tk%2
      banks 4-5: y context 0, banks 6-7: y context 1 (AV accumulators,
        8 groups of 65 cols, bank-packed 2+2)
      filler matmuls (projections, c_proj) borrow the idle y context.
  - AV(p) (phase p's attention-weight x V) is emitted 2 phases later
    (compressed near the end: {2:[0],3:[1],4:[2],5:[3,4],6:[5],7:[6]} and
    phase 7 carries its own AV lagged 2 iterations), so the A-phase
    projections spread across phases 0-2 and the tail stays short.
  - Deep et buffering (34 tiles) decouples ACT from PE.
"""

import contextlib

import numpy as np

import concourse.bass as bass
import concourse.mybir as mybir
import concourse.tile as tile
from concourse import bass_utils

F32 = mybir.dt.float32
BF16 = mybir.dt.bfloat16
EXP = mybir.ActivationFunctionType.Exp
MULT = mybir.AluOpType.mult


def legalize_waits(nc, max_waits=1, strip_self=False):
    """Walrus codegen rejects instructions carrying more than one sync wait.
    Split extra waits into preceding same-engine NoOps at the BIR-JSON level
    and pin the serialized module on the nc object.

    strip_self: drop waits on the instruction's own engine-completion
    semaphore. TESTED ON HW: BREAKS CORRECTNESS (rms 1.98) — same-engine
    waits are load-bearing on real silicon. Keep False."""
    import json as _json
    d = _json.loads(nc.to_json_bytes())
    ctr = 0
    for fn in d.get("functions", []):
        for blk in fn.get("blocks", []) or []:
            insts = blk.get("instructions")
            if not insts:
                continue
            out = []
            for inst in insts:
                si = inst.get("sync_info")
                waits = (si or {}).get("on_wait") or []
                if (strip_self and si and waits and inst["opcode"] not in
                        ("Drain", "NoOp") and len(waits) > 1):
                    own = f"{inst['engine']}_"
                    kept = [w for w in waits
                            if not str(w.get("ant_name", "")).startswith(own)]
                    if kept:  # never leave an instruction waitless here
                        si["on_wait"] = waits = kept
                if len(waits) > max_waits:
                    keep, extra = waits[:max_waits], waits[max_waits:]
                    for w in extra:
                        ctr += 1
                        out.append({
                            "debug": inst.get("debug", 0),
                            "engine": inst["engine"],
                            "ins": [],
                            "outs": [],
                            "name": f"I-wsplit-{ctr}",
                            "opcode": "NoOp",
                            "sync_info": {"on_wait": [w], "on_update": []},
                        })
                    si["on_wait"] = keep
                out.append(inst)
            blk["instructions"] = out
    raw = _json.dumps(d).encode()
    nc.to_json_bytes = lambda: raw
    return nc


B, T_FULL, C_FULL = 2, 2048, 1024
H_GLOBAL = 16
D = 64
N_CORES = 8
HL = 4
CLOC = HL * D
DE = D + 1


def emit_mha_kernel(tc, out, xT, wqk, wv, wp, T, C):
    nc = tc.nc

    CT = C // 128
    TT = T // 128
    KK = CLOC // 128
    QB = min(512, T)
    NQB = T // QB
    QT = QB // 128
    OSUB = min(512, C)
    OB = C // OSUB
    NPH = 2 * NQB

    stack = contextlib.ExitStack()
    persist = stack.enter_context(tc.tile_pool(name="persist", bufs=1))
    psum = stack.enter_context(tc.tile_pool(name="psum", bufs=1, space="PSUM"))
    rec_pool = stack.enter_context(tc.tile_pool(name="rec_pool", bufs=4))
    out_pool = stack.enter_context(tc.tile_pool(name="out_pool", bufs=4))
    et_pool = stack.enter_context(tc.tile_pool(name="et_pool", bufs=2 * TT + 2))

    # ---- persistent SBUF (bf16 operands) ----
    xt_sb = persist.tile([128, CT * T], BF16, name="xt_sb")
    wqk_sb = persist.tile([128, CT * 2 * CLOC], BF16, name="wqk_sb")
    wv_sb = persist.tile([128, CT * CLOC], BF16, name="wv_sb")
    wp_sb = persist.tile([128, KK * C], BF16, name="wp_sb")
    qk_sb = persist.tile([128, 4 * T], BF16, name="qk_sb")
    v_sb = persist.tile([128, TT * HL * DE], BF16, name="v_sb")
    yT_sb = persist.tile([128, KK * T], BF16, name="yT_sb")
    yn_sb = persist.tile([128, 2 * QT * CLOC], BF16, name="yn_sb")

    # ---- manual PSUM layout ----
    ps = psum.tile([128, 4096], F32, name="ps")

    def sc_ap(tk):
        s = tk % 2
        return ps[:, s * 1024:(s + 1) * 1024]

    def y_ap(ctx):
        return ps[:, 2048 + ctx * 1024: 2048 + (ctx + 1) * 1024]

    def yoff(t, i):
        return (t // 2) * 512 + (t % 2) * 260 + i * DE

    # filler PSUM: borrow the idle y context's banks, rotating halves
    state = {"fctx": 1, "half": 0}

    def filler_psum(ncols):
        if state.get("use_sc"):
            # final-tail mode: the QK/exp stream is done, sc banks are free
            h = state["half"]
            state["half"] = (h + 1) % 4
            return ps[:, h * 512: h * 512 + ncols]
        ctx = state["fctx"]
        assert ctx is not None, "PSUM filler emitted in a phase with no free context"
        h = state["half"]
        state["half"] ^= 1
        base = 2048 + ctx * 1024 + h * 512
        return ps[:, base: base + ncols]

    mm = nc.tensor.matmul

    # ---- input DMAs: wqk and xT block 0 in ct-halves (first A1 chunks
    # pipeline against arrival), then wv, remaining xT blocks, wp.
    XB = min(512, T)
    H2 = CT // 2
    for h in range(2):
        c0, c1 = h * H2, (h + 1) * H2
        nc.sync.dma_start(
            wqk_sb[:, c0 * 2 * CLOC: c1 * 2 * CLOC].rearrange(
                "p (k f) -> p k f", k=H2),
            wqk[c0 * 128: c1 * 128, :].rearrange("(k p) f -> p k f", p=128),
        )
        nc.sync.dma_start(
            xt_sb[:].rearrange("p (k f) -> p k f", k=CT)[:, c0:c1, 0:XB],
            xT[c0 * 128: c1 * 128, 0:XB].rearrange("(k p) f -> p k f", p=128),
        )
    nc.sync.dma_start(
        wv_sb[:].rearrange("p (k f) -> p k f", k=CT),
        wv[:].rearrange("(k p) f -> p k f", p=128),
    )
    for blk in range(1, T // XB):
        nc.sync.dma_start(
            xt_sb[:].rearrange("p (k f) -> p k f", k=CT)[:, :, blk * XB:(blk + 1) * XB],
            xT[:, blk * XB:(blk + 1) * XB].rearrange("(k p) f -> p k f", p=128),
        )
    nc.sync.dma_start(
        wp_sb[:].rearrange("p (k f) -> p k f", k=KK),
        wp[:].rearrange("(k p) f -> p k f", p=128),
    )

    # ones column in v (device-side)
    nc.gpsimd.memset(
        v_sb[:].rearrange("p (t h e) -> p t h e", t=TT, h=HL)[:, :, :, D:DE], 1.0
    )
    # warm the ACT exp table during the DMA wait (saves ~2.7us on the
    # first real exp): tiny dummy exp on a memset scratch.
    nc.vector.memset(yn_sb[0:1, 0:8], 0.0)
    nc.scalar.activation(yn_sb[0:1, 0:8], yn_sb[0:1, 0:8], EXP, scale=1.0)

    # ---------------- deferred PE work (fillers) ----------------
    # Items: (est_cycles, emit_fn, deadline, needs_psum). Deadline is a
    # fractional phase index: the item MUST be emitted (program order)
    # before that point. The drain scans the whole queue for forced items,
    # so deadlines need not be monotone; PSUM-needing items are skipped
    # (order preserved among themselves) when no y context is free.
    fillers = []

    def push(item, deadline, needs_psum=True):
        cyc, fn = item
        fillers.append([cyc, fn, deadline, needs_psum])

    def emit_fill(budget, now=1e30):
        # everything up to the DEEPEST forced item must go (program order);
        # afterwards keep popping while budget lasts.
        last = -1
        for idx, it in enumerate(fillers):
            if it[2] <= now:
                last = idx
        i = 0
        while i < len(fillers):
            cyc, fn, dl, np_ = fillers[i]
            must = i <= last
            if not must and budget <= 0:
                break
            if np_ and state["fctx"] is None:
                # safe to leave in place: later items never write what a
                # skipped item reads (tr(tb,tq)->cproj(tb,tq) stays FIFO).
                assert dl > now, "deadline-forced PSUM filler, no free ctx"
                i += 1
                continue
            fillers.pop(i)
            last -= 1
            fn()
            budget -= cyc

    BLK = min(256, T)
    NB = T // BLK

    def a1_chunk(ft, blk):
        """qkT block for f-tile ft (0=q01,1=q23,2=k01,3=k23) -> qk_sb bf16."""
        def fn():
            pst = filler_psum(BLK)
            for ct in range(CT):
                lhsT = wqk_sb[:, ct * 2 * CLOC + ft * 128: ct * 2 * CLOC + (ft + 1) * 128]
                t0 = ct * T + blk * BLK
                mm(pst, lhsT, xt_sb[:, t0:t0 + BLK],
                   start=(ct == 0), stop=(ct == CT - 1))
            nc.vector.tensor_copy(
                qk_sb[:, ft * T + blk * BLK: ft * T + (blk + 1) * BLK], pst
            )
        return (CT * BLK, fn)

    def a2_chunk(tt):
        """v natural tile tt -> v_sb bf16 (strided copy keeps ones column)."""
        def fn():
            pst = filler_psum(CLOC)
            for ct in range(CT):
                mm(pst,
                   xt_sb[:, ct * T + tt * 128: ct * T + (tt + 1) * 128],
                   wv_sb[:, ct * CLOC:(ct + 1) * CLOC],
                   start=(ct == 0), stop=(ct == CT - 1))
            dst = v_sb[:, tt * HL * DE:(tt + 1) * HL * DE].rearrange(
                "p (h e) -> p h e", h=HL)[:, :, 0:D]
            nc.vector.tensor_copy(dst, pst.rearrange("p (h d) -> p h d", h=HL))
        return (CT * CLOC, fn)

    def tr_chunk(tb, tq, eng=None):
        """Transpose yn [128tq, CLOC] -> yT_sb [ch, tq] via DMA XBAR (bf16)."""
        def fn():
            e = eng if eng is not None else nc.sync
            yn0 = ((tb % 2) * QT + tq) * CLOC
            tt = tb * QT + tq
            for kk in range(KK):
                e.dma_start(
                    yT_sb[:, kk * T + tt * 128: kk * T + (tt + 1) * 128],
                    yn_sb[:, yn0 + kk * 128: yn0 + (kk + 1) * 128],
                    transpose=True,
                )
        return (100, fn)

    def cproj_chunk(tt, tail=False, split_dma=None):
        """c_proj row tile tt -> out[tt*128:(tt+1)*128, :]. Tail chunks
        evacuate + issue their DMA on ACT (idle after the last exp)."""
        def fn():
            osb = out_pool.tile([128, C], BF16, name="osb")
            for ob in range(OB):
                ops = filler_psum(OSUB)
                for kk in range(KK):
                    mm(ops,
                       yT_sb[:, kk * T + tt * 128: kk * T + (tt + 1) * 128],
                       wp_sb[:, kk * C + ob * OSUB: kk * C + (ob + 1) * OSUB],
                       start=(kk == 0), stop=(kk == KK - 1))
                if tail and ob % 2 == 1:
                    nc.scalar.activation(osb[:, ob * OSUB:(ob + 1) * OSUB], ops,
                                         mybir.ActivationFunctionType.Copy)
                else:
                    nc.vector.tensor_copy(osb[:, ob * OSUB:(ob + 1) * OSUB], ops)
                if tail or split_dma:
                    # per-ob DMA: the transfer starts before the next evac
                    nc.sync.dma_start(
                        out[tt * 128:(tt + 1) * 128, ob * OSUB:(ob + 1) * OSUB],
                        osb[:, ob * OSUB:(ob + 1) * OSUB])
            if not (tail or split_dma):
                nc.sync.dma_start(out[tt * 128:(tt + 1) * 128, :], osb[:])
        return (OB * KK * OSUB + 300, fn)

    # ---------------- attention phases ----------------
    et_tiles = {}

    def av_tk(p, tk):
        """One tk slice of AV for phase p (head pair p%2 of block p//2)."""
        g = p % 2
        yb = y_ap(p % 2)
        et = et_tiles[(p, tk)]
        for t in range(QT):
            for i in range(2):
                hh = 2 * g + i
                mm(yb[:, yoff(t, i):yoff(t, i) + DE],
                   et[:, i * QB + t * 128: i * QB + t * 128 + 128],
                   v_sb[:, tk * HL * DE + hh * DE: tk * HL * DE + (hh + 1) * DE],
                   start=(tk == 0 and i == 0 and t % 2 == 0),
                   stop=(tk == TT - 1 and i == 1 and t % 2 == 1),
                   skip_group_check=True)

    def norm(p):
        """Normalize phase p's AV output into yn_sb (and free its context).

        Per tq-tile: one paired reciprocal [128,2] (both heads' denominators
        via the stride-65 head axis) + one scalar_tensor_tensor with the
        reciprocals broadcast along the free dim (stride-0)."""
        g = p % 2
        tb = p // 2
        ybt = y_ap(p % 2)
        yn_base = (tb % 2) * QT * CLOC
        for t in range(QT):
            pair = ybt[:, yoff(t, 0):yoff(t, 0) + 2 * DE].rearrange(
                "p (h e) -> p h e", h=2)
            rec = rec_pool.tile([128, 2], F32, name="rec")
            with nc.allow_low_precision(reason="softmax denominators"):
                nc.vector.reciprocal(
                    rec[:].rearrange("p (h e) -> p h e", h=2),
                    pair[:, :, D:DE])
            c0 = yn_base + t * CLOC + 2 * g * D
            nc.vector.scalar_tensor_tensor(
                yn_sb[:, c0: c0 + 2 * D].rearrange("p (h r) -> p h r", h=2),
                pair[:, :, 0:D], 1.0,
                rec[:, :, None].broadcast_to([128, 2, D]),
                mybir.AluOpType.bypass, MULT,
            )

    # AV(p) emission phase: compressed at the end so the tail stays short.
    run_map = {2: [0], 3: [1], 4: [2], 5: [3, 4], 6: [5], 7: [6]}
    if NPH == 2:  # tiny-T testing config
        run_map = {1: [0]}

    def qk_exp_phase(pi, tb, g):
        budget = 1544 if pi <= 1 else (600 if (pi == 5 or pi == NPH - 1) else 1024)
        streams = run_map.get(pi, [])
        self_inphase = (pi == NPH - 1)
        busy = {p % 2 for p in streams} | ({pi % 2} if self_inphase else set())
        free = [c for c in (0, 1) if c not in busy]
        state["fctx"] = free[0] if free else None
        qcol = g * T
        kcol = (2 + g) * T
        for tk in range(TT):
            emit_fill(0, pi + tk / TT)
            sc = sc_ap(tk)
            for i in range(2):
                p0 = i * 64
                mm(sc[:, i * QB:(i + 1) * QB],
                   qk_sb[p0:p0 + 64, kcol + tk * 128: kcol + (tk + 1) * 128],
                   qk_sb[p0:p0 + 64, qcol + tb * QB: qcol + (tb + 1) * QB],
                   start=True, stop=True)
            et = et_pool.tile([128, 2 * QB], BF16, name="et")
            et_tiles[(pi, tk)] = et
            nc.scalar.activation(et[:], sc, EXP, scale=float(1.0 / np.sqrt(D)))
            for p in streams:
                av_tk(p, tk)
            if self_inphase and tk >= 2:
                av_tk(pi, tk - 2)
            emit_fill(budget, pi + tk / TT)
        emit_fill(0, pi + 0.995)
        if self_inphase:
            av_tk(pi, TT - 2)
            av_tk(pi, TT - 1)
        for p in streams:
            norm(p)
        if self_inphase:
            norm(pi)

    # ---------------- emission schedule ----------------
    # Pre-phase: q01 chunks covering tb0 (cols 0:512) + k01 chunk 0.
    NBQ = max(1, 512 // BLK)   # chunks per 512-col q block
    for ft, blk in [(0, b) for b in range(NBQ)] + [(2, 0)]:
        _, fn = a1_chunk(ft, blk)
        fn()

    # k01 chunk j feeds QK(phase0, tk=2j); pair1 q/k feed phase 1.
    for j in range(1, NB):
        push(a1_chunk(2, j), 2 * j / TT)
    pair1_base = 1.0 if NPH > 2 else 0.49
    for b_ in range(NBQ):
        push(a1_chunk(1, b_), pair1_base)
    for j in range(NB):
        push(a1_chunk(3, j), pair1_base + (2 * j / TT if NPH > 2 else 0.0))
    for tt in range(TT):
        push(a2_chunk(tt), 2.0 + tt / TT if NPH > 2 else 0.99)

    phases = [(tb, g) for tb in range(NQB) for g in range(2)]
    for pi, (tb, g) in enumerate(phases):
        qk_exp_phase(pi, tb, g)
        if g == 0 and tb + 1 < NQB:
            # q blocks for tb+1 (both pairs); deadlines avoid phase 5
            # (no free y context there).
            dls = {1: (2.0, 3.0), 2: (4.0, 4.5), 3: (6.0, 6.2)}[tb + 1]
            for b_ in range(NBQ):
                push(a1_chunk(0, (tb + 1) * NBQ + b_), dls[0])
                push(a1_chunk(1, (tb + 1) * NBQ + b_), dls[1])
        # tr/cproj for block tbd once BOTH its pair norms have been emitted
        # (norm(2tbd+1) lands at the end of run phase of stream 2tbd+1).
        done_tb = {3: 0, 5: 1, 6: 2, 7: 3}.get(pi) if NPH > 2 else (
            0 if pi == NPH - 1 else None)
        if done_tb is not None:
            dl_tr, dl_cp = ({0: (4.0, 4.5), 1: (6.0, 6.5), 2: (7.0, 99.0),
                             3: (99.0, 99.0)}[done_tb] if NPH > 2
                            else (99.0, 99.0))
            eng = nc.scalar if done_tb == NQB - 1 else None  # ACT idle in tail
            for tq in range(QT):
                push(tr_chunk(done_tb, tq, eng), dl_tr, needs_psum=False)
                push(cproj_chunk(done_tb * QT + tq, done_tb == NQB - 1), dl_cp)

    state["fctx"] = 0
    emit_fill(1 << 30)

    stack.close()


def build_nc(T=T_FULL, C=C_FULL):
    nc = bass.Bass("TRN2")
    xT = nc.dram_tensor("xT", [C, T], BF16, kind="ExternalInput")
    wqk = nc.dram_tensor("wqk", [C, 2 * CLOC], BF16, kind="ExternalInput")
    wv = nc.dram_tensor("wv", [C, CLOC], BF16, kind="ExternalInput")
    wp = nc.dram_tensor("wp", [CLOC, C], BF16, kind="ExternalInput")
    out = nc.dram_tensor("out", [T, C], BF16, kind="ExternalOutput")
    with tile.TileContext(nc) as tc:
        emit_mha_kernel(tc, out[:], xT[:], wqk[:], wv[:], wp[:], T, C)
    return legalize_waits(nc)


def make_in_maps(x, W_attn, W_proj):
    import ml_dtypes
    bf16 = ml_dtypes.bfloat16
    C = x.shape[2]
    in_maps = []
    for core in range(N_CORES):
        b, hg = divmod(core, N_CORES // B)
        s0, s1 = hg * CLOC, (hg + 1) * CLOC
        Wq = W_attn[s0:s1, :]
        Wk = W_attn[C + s0:C + s1, :]
        Wv = W_attn[2 * C + s0:2 * C + s1, :]
        in_maps.append({
            "xT": np.ascontiguousarray(x[b].T).astype(bf16),
            "wqk": np.ascontiguousarray(np.concatenate([Wq, Wk], 0).T).astype(bf16),
            "wv": np.ascontiguousarray(Wv.T).astype(bf16),
            "wp": np.ascontiguousarray(W_proj[:, s0:s1].T).astype(bf16),
        })
    return in_maps


_CACHED_NC = None


def kernel(x, W_attn, W_proj, b_proj, _trace=False):
    global _CACHED_NC
    x = np.asarray(x, dtype=np.float32)
    W_attn = np.asarray(W_attn, dtype=np.float32)
    W_proj = np.asarray(W_proj, dtype=np.float32)
    b_proj = np.asarray(b_proj, dtype=np.float32)

    if _CACHED_NC is None:
        _CACHED_NC = build_nc(T=x.shape[1], C=x.shape[2])
    nc = _CACHED_NC

    in_maps = make_in_maps(x, W_attn, W_proj)
    res = bass_utils.run_bass_kernel_spmd(
        nc, in_maps, core_ids=list(range(N_CORES)), trace=_trace,
    )
    parts = [np.asarray(r["out"], dtype=np.float32) for r in res.results]
    G = N_CORES // B
    out = np.stack(
        [np.sum(parts[b * G:(b + 1) * G], axis=0) + b_proj for b in range(B)], axis=0
    ).astype(np.float32)
    if _trace:
        return out, res
    return out


if __name__ == "__main__":
    nc = build_nc()
    print("built OK")
